# revision 8
# baseline (speedup 1.0000x reference)
"""Trainium2 Bass kernel for the logic-model log-likelihood (v3).

Changes vs v2 (driven by the v2 NTFF trace):
  - No dummy activation and no const-block bias columns: the framework's
    own memset consts serve activation biases, so nothing on the Act
    engine waits for the const DMA. The single (rewritten) act-table
    load sits first in the Act program and runs at t~0.
  - Query times are broadcast as fp16 (half the DMA bytes; host supplies
    the fp16 copy), and compares run as 24 tensor_scalar ops in DVE 2x
    mode, interleaved per-sample with the matvec matmuls.
  - DMAs: small inputs first, split across SP/Pool/DVE issue queues.
  - Integral ALU chain runs on the otherwise-idle GpSimd engine.
  - Epilogue: eqd is pre-masked with the valid mask, so dead cells give
    ln(b); the host subtracts sum((384-V)*ln(b)) exactly. Path after the
    last matmul is just mult -> Ln(accum) -> 2 tiny matmuls -> DMA (from
    PSUM directly).
"""
import sys

import numpy as np

sys.path.insert(0, "/opt/trn_rl_repo")

import concourse.bacc as bacc
import concourse.mybir as mybir
from concourse import tile
from concourse.bass_utils import run_bass_kernel_spmd

F32 = mybir.dt.float32
F16 = mybir.dt.float16
I32 = mybir.dt.int32
BF16 = mybir.dt.bfloat16
AF = mybir.ActivationFunctionType
ALU = mybir.AluOpType

N_CORES = 8
S, P, E = 64, 3, 128
SC = S // N_CORES          # samples per core
ROWS = SC * P              # 24 (s,p) rows per core
DECAY, RES, TOL = 0.8, 0.03, 0.1
G = 1667                   # len(np.arange(0, 50, 0.03))
INV1MR = float(1.0 / (1.0 - np.exp(-DECAY * RES)))
E2C = float(np.exp(-DECAY * G * RES))
BODY = np.array([[0, 1, 1], [1, 0, 0], [1, 0, 0]], dtype=np.float32)

# natural_log_exp_and_others: exp, ln, copy, relu in one table
_ACT_SET_ALL = 6


def _cblk():
    # [128, 218]: eye24 | ones | -D*RES | bdiag[192]
    cb = np.zeros((128, 218), np.float32)
    cb[0:ROWS, 0:ROWS] = np.eye(ROWS)
    cb[:, ROWS] = 1.0
    cb[:, ROWS + 1] = -DECAY * RES
    bd = np.zeros((SC, ROWS), np.float32)
    for s in range(SC):
        bd[s, 3 * s:3 * s + 3] = 1.0
    cb[:, 26:26 + SC * ROWS] = bd.reshape(1, SC * ROWS)
    return cb


def _rdiag():
    rd = np.zeros((ROWS, P, E), np.float32)
    for r in range(ROWS):
        rd[r, r % 3, :] = 1.0
    return rd.reshape(ROWS, P * E)


def _build_nc():
    nc = bacc.Bacc(None, target_bir_lowering=False)
    em_d = nc.dram_tensor("em", [ROWS, 2 * E + 2], F32, kind="ExternalInput")
    t16_d = nc.dram_tensor("t16", [ROWS, E], F16, kind="ExternalInput")
    out_d = nc.dram_tensor("out", [ROWS, 3], F32, kind="ExternalOutput")
    cb_d = nc.inline_tensor(_cblk(), "cblk")
    rd_d = nc.inline_tensor(_rdiag(), "rdiag")
    t16_flat = t16_d[:].rearrange("a b -> (a b)")

    with tile.TileContext(nc) as tc:
        with (
            tc.tile_pool(name="const", bufs=1) as cpool,
            tc.tile_pool(name="inp", bufs=1) as ipool,
            tc.tile_pool(name="tq", bufs=1) as qpool,
            tc.tile_pool(name="cmp", bufs=1) as cmpool,
            tc.tile_pool(name="work", bufs=1) as wpool,
            tc.tile_pool(name="psT", bufs=1, space="PSUM") as psT,
            tc.tile_pool(name="psA", bufs=1, space="PSUM") as psA,
            tc.tile_pool(name="psK", bufs=1, space="PSUM") as psK,
            tc.tile_pool(name="psI", bufs=1, space="PSUM") as psI,
            tc.tile_pool(name="psD", bufs=1, space="PSUM") as psD,
        ):
            # ---- DMAs: merged input first; const + 2nd bcast on GpSimd ----
            im = ipool.tile([ROWS, 2 * E + 2], F32, tag="im")
            nc.sync.dma_start(im[:], em_d[:])
            tq16 = qpool.tile([128, SC * P * E], F16, tag="tq16")
            nc.sync.dma_start(
                tq16[:, 0:4 * P * E],
                t16_flat[0:4 * P * E].partition_broadcast(128))
            cblk = cpool.tile([128, 218], F32, tag="cblk")
            nc.gpsimd.dma_start(cblk[:], cb_d[:])
            nc.gpsimd.dma_start(
                tq16[:, 4 * P * E:8 * P * E],
                t16_flat[4 * P * E:8 * P * E].partition_broadcast(128))
            rdiag_t = cpool.tile([ROWS, P * E], F32, tag="rdiag")
            nc.gpsimd.dma_start(rdiag_t[:], rd_d[:])

            t_sb = im[:, 0:E]
            m_sb = im[:, E:2 * E]
            w_col = im[:, 2 * E:2 * E + 1]
            b_col = im[:, 2 * E + 1:2 * E + 2]
            eye24 = cblk[0:ROWS, 0:ROWS]
            ones_col = cblk[:, ROWS:ROWS + 1]
            nc.const_aps.aps[(F32, -DECAY * RES)] = \
                cblk[:, ROWS + 1:ROWS + 2]
            bdiag = cblk[:, 26:26 + SC * ROWS]
            rdiag = rdiag_t[:]

            # ---- prep ----
            aexp = wpool.tile([ROWS, E], F32, tag="aexp")
            nc.scalar.activation(aexp[:], t_sb, AF.Exp, scale=DECAY)
            a_sb = wpool.tile([ROWS, E], F32, tag="a_sb")
            nc.vector.tensor_mul(a_sb[:], aexp[:], m_sb)

            t_ps = psT.tile([128, ROWS], F32, tag="t_ps")
            nc.tensor.transpose(t_ps[:], t_sb, eye24)
            tp_T = wpool.tile([128, ROWS], F32, tag="tp_T")  # t^T + TOL
            nc.vector.tensor_scalar(tp_T[:], t_ps[:], TOL, None, ALU.add)

            a_ps = psA.tile([128, ROWS], F32, tag="a_ps")
            nc.tensor.transpose(a_ps[:], a_sb[:], eye24)
            a_T = wpool.tile([128, ROWS + 2], BF16, tag="a_T")
            nc.vector.tensor_copy(a_T[:, 0:ROWS], a_ps[:])
            nc.vector.memset(a_T[:, ROWS:ROWS + 2], 0.0)

            # S1 gather: col 3s <- a[s,1]; cols 3s+1,3s+2 <- a[s,0]
            s1 = wpool.tile([128, ROWS + 1], BF16, tag="s1")
            nc.vector.tensor_copy(s1[:, 0:ROWS:3], a_T[:, 1:ROWS:3])
            nc.vector.tensor_copy(
                s1[:, 1:ROWS + 1].rearrange("p (a b) -> p a b", b=3)[:, :, 0:2],
                a_T[:, 0:ROWS:3][:, :, None].broadcast_to([128, SC, 2]))
            # block-diag masked stationaries [128, 192]
            s1_big = wpool.tile([128, SC * ROWS], BF16, tag="s1_big")
            nc.gpsimd.tensor_mul(
                s1_big[:].rearrange("p (a b) -> p a b", b=ROWS),
                s1[:, 0:ROWS][:, None, :].broadcast_to([128, SC, ROWS]),
                bdiag.rearrange("p (a b) -> p a b", b=ROWS))
            s2_big = wpool.tile([128, SC * ROWS], BF16, tag="s2_big")
            nc.gpsimd.tensor_mul(
                s2_big[:].rearrange("p (a b) -> p a b", b=ROWS),
                a_T[:, 2:ROWS + 2][:, None, :].broadcast_to([128, SC, ROWS]),
                bdiag.rearrange("p (a b) -> p a b", b=ROWS))

            # valid mask: zero the event-0 column (after a_sb consumed m_sb)
            nc.vector.memset(im[:, E:E + 1], 0.0)

            # eqd_m[(s,h), 128h+e] = exp(-D*t[s,h,e]) * valid, 0 off-diagonal
            eqd = wpool.tile([ROWS, P * E], F32, tag="eqd")
            nc.scalar.activation(
                eqd[:].rearrange("p (a b) -> p a b", b=E),
                t_sb[:, None, :].broadcast_to([ROWS, P, E]),
                AF.Exp, scale=-DECAY)
            vd = wpool.tile([ROWS, P * E], F32, tag="vd")
            nc.gpsimd.tensor_mul(
                vd[:].rearrange("p (a b) -> p a b", b=E),
                m_sb[:, None, :].broadcast_to([ROWS, P, E]),
                rdiag.rearrange("p (a b) -> p a b", b=E))
            nc.gpsimd.tensor_mul(eqd[:], eqd[:], vd[:])

            # ---- integral ALU chain on GpSimd (idle engine) ----
            u = wpool.tile([128, ROWS], F32, tag="u")
            nc.vector.tensor_scalar(u[:], tp_T[:], 1.0 / RES, None, ALU.mult)
            ci_t = wpool.tile([128, ROWS], I32, tag="ci_t")
            nc.vector.tensor_copy(ci_t[:], u[:])
            cf_t = wpool.tile([128, ROWS], F32, tag="cf_t")
            nc.vector.tensor_copy(cf_t[:], ci_t[:])
            gt_t = wpool.tile([128, ROWS], F32, tag="gt_t")
            nc.vector.tensor_tensor(gt_t[:], cf_t[:], u[:], ALU.is_gt)
            nc.vector.tensor_sub(gt_t[:], cf_t[:], gt_t[:])  # floor(u)
            ie = wpool.tile([128, ROWS], F32, tag="ie")
            nc.scalar.activation(ie[:], gt_t[:], AF.Exp, scale=-DECAY * RES,
                                 bias=-DECAY * RES)
            nc.vector.tensor_scalar(ie[:], ie[:], E2C, 0.0, ALU.subtract,
                                    ALU.max)
            cm = wpool.tile([128, ROWS], F32, tag="cm")
            nc.vector.tensor_mul(cm[:], ie[:], a_ps[:])
            kint_ps = psI.tile([ROWS, 1], F32, tag="kint")
            nc.tensor.matmul(kint_ps[:], cm[:], ones_col, start=True, stop=True)


            # ---- compares + matvecs, interleaved per sample ----
            # per-sample 512 col block of C: [c1 | c0a c0b | c2]
            call = cmpool.tile([128, SC * 4 * E], BF16, tag="call")
            kk_ps = psK.tile([ROWS, P * E], F32, tag="kk")

            for s in range(SC):
                base = 4 * E * s
                q0 = tq16[:, P * E * s:P * E * s + E]        # pred-0 queries
                q12 = tq16[:, P * E * s + E:P * E * (s + 1)]  # pred-1,2 queries
                nc.vector.tensor_scalar(
                    call[:, base:base + E], q0,
                    tp_T[:, 3 * s + 1:3 * s + 2], None, ALU.is_gt)
                nc.vector.tensor_scalar(
                    call[:, base + E:base + 3 * E], q12,
                    tp_T[:, 3 * s:3 * s + 1], None, ALU.is_gt)
                nc.vector.tensor_scalar(
                    call[:, base + 3 * E:base + 4 * E], q0,
                    tp_T[:, 3 * s + 2:3 * s + 3], None, ALU.is_gt)
                nc.tensor.matmul(
                    kk_ps[:], s1_big[:, ROWS * s:ROWS * (s + 1)],
                    call[:, base:base + 3 * E],
                    start=(s == 0), stop=False, skip_group_check=True)
                nc.tensor.matmul(
                    kk_ps[:, 0:E], s2_big[:, ROWS * s:ROWS * (s + 1)],
                    call[:, base + 3 * E:base + 4 * E],
                    start=False, stop=(s == SC - 1), skip_group_check=True)

            # ---- epilogue: keq -> ln(w*keq+b) with free accum ----
            keq = wpool.tile([ROWS, P * E], F32, tag="keq")
            lnr = wpool.tile([ROWS, P * E], F32, tag="lnr")
            acc = wpool.tile([ROWS, 3], F32, tag="acc")
            nc.vector.tensor_copy(acc[:, 2:3], kint_ps[:])
            nc.vector.tensor_mul(keq[:, E:], kk_ps[:, E:], eqd[:, E:])
            nc.scalar.activation(lnr[:, E:], keq[:, E:], AF.Ln, bias=b_col,
                                 scale=w_col, accum_out=acc[:, 0:1])
            nc.vector.tensor_mul(keq[:, 0:E], kk_ps[:, 0:E], eqd[:, 0:E])
            nc.scalar.activation(lnr[:, 0:E], keq[:, 0:E], AF.Ln, bias=b_col,
                                 scale=w_col, accum_out=acc[:, 1:2])
            nc.scalar.dma_start(out_d[:], acc[:])

    nc.compile()
    _unify_act_tables(nc)
    return nc


def _unify_act_tables(nc):
    for blk in nc.m.functions[0].blocks:
        loads = [i for i in blk.instructions
                 if isinstance(i, mybir.InstLoadActFuncSet)]
        if not loads:
            continue
        loads[0].act_func_set_id = _ACT_SET_ALL
        for ins in loads[1:]:
            blk.instructions.remove(ins)


_NC = None


def _get_nc():
    global _NC
    if _NC is None:
        _NC = _build_nc()
    return _NC


def make_in_maps(event_times, event_mask, base, weight):
    et = np.ascontiguousarray(np.asarray(event_times, np.float32))
    mk = np.ascontiguousarray(np.asarray(event_mask, np.float32))
    w = np.asarray(weight, np.float32).reshape(P)
    b = np.asarray(base, np.float32).reshape(P)
    in_maps = []
    for c in range(N_CORES):
        et_c = et[c * SC:(c + 1) * SC].reshape(ROWS, E)
        em = np.zeros((ROWS, 2 * E + 2), np.float32)
        em[:, 0:E] = et_c
        em[:, E:2 * E] = mk[c * SC:(c + 1) * SC].reshape(ROWS, E)
        em[:, 2 * E] = np.tile(w, SC)
        em[:, 2 * E + 1] = np.tile(b, SC)
        in_maps.append({"em": em, "t16": et_c.astype(np.float16)})
    return in_maps


def host_const(event_mask, base):
    """-RES*G*S*sum(b)  minus the ln(b) contributions of dead cells."""
    b = np.asarray(base, np.float64).reshape(P)
    mk = np.asarray(event_mask, np.float64)
    v_cnt = mk[:, :, 1:].sum(axis=2)              # [S, P] valid counts
    junk = ((P * E - v_cnt) * np.log(b)[None, :]).sum()
    return float(-RES * G * S * b.sum() - junk)


LAST_RESULT = None


def kernel(event_times, event_mask, base, weight, T_max=50, _trace=False, **_):
    global LAST_RESULT
    nc = _get_nc()
    in_maps = make_in_maps(event_times, event_mask, base, weight)
    kwargs = {}
    if _trace:
        kwargs = dict(trace=True, trace_cores=list(range(N_CORES)))
    res = run_bass_kernel_spmd(nc, in_maps, core_ids=list(range(N_CORES)),
                               **kwargs)
    LAST_RESULT = res
    w = np.asarray(weight, np.float64).reshape(P)
    v = -RES * INV1MR * (BODY.T @ w)          # [P]
    v24 = np.tile(v, SC)
    total = np.float64(0.0)
    for r in res.results:
        out = np.asarray(r["out"], np.float64)
        total += out[:, 0].sum() + out[:, 1].sum() + (out[:, 2] * v24).sum()
    total += host_const(event_mask, base)
    return np.asarray(total, dtype=np.float32)


# revision 9
# speedup vs baseline: 1.0462x; 1.0462x over previous
"""Trainium2 Bass kernel for the logic-model log-likelihood (v3).

Changes vs v2 (driven by the v2 NTFF trace):
  - No dummy activation and no const-block bias columns: the framework's
    own memset consts serve activation biases, so nothing on the Act
    engine waits for the const DMA. The single (rewritten) act-table
    load sits first in the Act program and runs at t~0.
  - Query times are broadcast as fp16 (half the DMA bytes; host supplies
    the fp16 copy), and compares run as 24 tensor_scalar ops in DVE 2x
    mode, interleaved per-sample with the matvec matmuls.
  - DMAs: small inputs first, split across SP/Pool/DVE issue queues.
  - Integral ALU chain runs on the otherwise-idle GpSimd engine.
  - Epilogue: eqd is pre-masked with the valid mask, so dead cells give
    ln(b); the host subtracts sum((384-V)*ln(b)) exactly. Path after the
    last matmul is just mult -> Ln(accum) -> 2 tiny matmuls -> DMA (from
    PSUM directly).
"""
import sys

import numpy as np

sys.path.insert(0, "/opt/trn_rl_repo")

import concourse.bacc as bacc
import concourse.mybir as mybir
from concourse import tile
from concourse.bass_utils import run_bass_kernel_spmd

F32 = mybir.dt.float32
F16 = mybir.dt.float16
I32 = mybir.dt.int32
BF16 = mybir.dt.bfloat16
AF = mybir.ActivationFunctionType
ALU = mybir.AluOpType

N_CORES = 8
S, P, E = 64, 3, 128
SC = S // N_CORES          # samples per core
ROWS = SC * P              # 24 (s,p) rows per core
DECAY, RES, TOL = 0.8, 0.03, 0.1
G = 1667                   # len(np.arange(0, 50, 0.03))
INV1MR = float(1.0 / (1.0 - np.exp(-DECAY * RES)))
E2C = float(np.exp(-DECAY * G * RES))
BODY = np.array([[0, 1, 1], [1, 0, 0], [1, 0, 0]], dtype=np.float32)

# natural_log_exp_and_others: exp, ln, copy, relu in one table
_ACT_SET_ALL = 6


def _cblk():
    # [128, 218]: eye24 | ones | -D*RES | bdiag[192]
    cb = np.zeros((128, 218), np.float32)
    cb[0:ROWS, 0:ROWS] = np.eye(ROWS)
    cb[:, ROWS] = 1.0
    cb[:, ROWS + 1] = -DECAY * RES
    bd = np.zeros((SC, ROWS), np.float32)
    for s in range(SC):
        bd[s, 3 * s:3 * s + 3] = 1.0
    cb[:, 26:26 + SC * ROWS] = bd.reshape(1, SC * ROWS)
    return cb


def _rdiag():
    rd = np.zeros((ROWS, P, E), np.float32)
    for r in range(ROWS):
        rd[r, r % 3, :] = 1.0
    return rd.reshape(ROWS, P * E)


def _build_nc():
    nc = bacc.Bacc(None, target_bir_lowering=False)
    em_d = nc.dram_tensor("em", [ROWS, 2 * E + 2], F32, kind="ExternalInput")
    t16_d = nc.dram_tensor("t16", [ROWS, E], F16, kind="ExternalInput")
    out_d = nc.dram_tensor("out", [ROWS, 2], F32, kind="ExternalOutput")
    cb_d = nc.inline_tensor(_cblk(), "cblk")
    rd_d = nc.inline_tensor(_rdiag(), "rdiag")
    t16_flat = t16_d[:].rearrange("a b -> (a b)")

    with tile.TileContext(nc) as tc:
        with (
            tc.tile_pool(name="const", bufs=1) as cpool,
            tc.tile_pool(name="inp", bufs=1) as ipool,
            tc.tile_pool(name="tq", bufs=1) as qpool,
            tc.tile_pool(name="cmp", bufs=1) as cmpool,
            tc.tile_pool(name="work", bufs=1) as wpool,
            tc.tile_pool(name="psT", bufs=1, space="PSUM") as psT,
            tc.tile_pool(name="psA", bufs=1, space="PSUM") as psA,
            tc.tile_pool(name="psK", bufs=1, space="PSUM") as psK,
            tc.tile_pool(name="psI", bufs=1, space="PSUM") as psI,
            tc.tile_pool(name="psD", bufs=1, space="PSUM") as psD,
        ):
            # ---- DMAs: broadcasts split by partition ranges so their
            # per-partition descriptors land on parallel queues ----
            im = ipool.tile([ROWS, 2 * E + 2], F32, tag="im")
            nc.sync.dma_start(im[:], em_d[:])
            tq16 = qpool.tile([128, SC * P * E], F16, tag="tq16")
            ha = 4 * P * E
            for lo, hi in ((0, 32), (32, 64)):
                nc.sync.dma_start(
                    tq16[lo:hi, 0:ha],
                    t16_flat[0:ha].partition_broadcast(hi - lo))
            for lo, hi in ((64, 96), (96, 128)):
                nc.scalar.dma_start(
                    tq16[lo:hi, 0:ha],
                    t16_flat[0:ha].partition_broadcast(hi - lo))
            cblk = cpool.tile([128, 218], F32, tag="cblk")
            nc.gpsimd.dma_start(cblk[:], cb_d[:])
            for lo, hi in ((0, 64), (64, 128)):
                nc.gpsimd.dma_start(
                    tq16[lo:hi, ha:2 * ha],
                    t16_flat[ha:2 * ha].partition_broadcast(hi - lo))
            rdiag_t = cpool.tile([ROWS, P * E], F32, tag="rdiag")
            nc.gpsimd.dma_start(rdiag_t[:], rd_d[:])

            t_sb = im[:, 0:E]
            m_sb = im[:, E:2 * E]
            w_col = im[:, 2 * E:2 * E + 1]
            b_col = im[:, 2 * E + 1:2 * E + 2]
            eye24 = cblk[0:ROWS, 0:ROWS]
            ones_col = cblk[:, ROWS:ROWS + 1]
            nc.const_aps.aps[(F32, -DECAY * RES)] = \
                cblk[:, ROWS + 1:ROWS + 2]
            bdiag = cblk[:, 26:26 + SC * ROWS]
            rdiag = rdiag_t[:]

            # ---- prep ----
            aexp = wpool.tile([ROWS, E], F32, tag="aexp")
            nc.scalar.activation(aexp[:], t_sb, AF.Exp, scale=DECAY)
            a_sb = wpool.tile([ROWS, E], F32, tag="a_sb")
            nc.vector.tensor_mul(a_sb[:], aexp[:], m_sb)

            t_ps = psT.tile([128, ROWS], F32, tag="t_ps")
            nc.tensor.transpose(t_ps[:], t_sb, eye24)
            tp_T = wpool.tile([128, ROWS], F32, tag="tp_T")  # t^T + TOL
            nc.vector.tensor_scalar(tp_T[:], t_ps[:], TOL, None, ALU.add)

            a_ps = psA.tile([128, ROWS], F32, tag="a_ps")
            nc.tensor.transpose(a_ps[:], a_sb[:], eye24)
            a_T = wpool.tile([128, ROWS + 2], BF16, tag="a_T")
            nc.vector.tensor_copy(a_T[:, 0:ROWS], a_ps[:])
            nc.vector.memset(a_T[:, ROWS:ROWS + 2], 0.0)

            # S1 gather: col 3s <- a[s,1]; cols 3s+1,3s+2 <- a[s,0]
            s1 = wpool.tile([128, ROWS + 1], BF16, tag="s1")
            nc.vector.tensor_copy(s1[:, 0:ROWS:3], a_T[:, 1:ROWS:3])
            nc.vector.tensor_copy(
                s1[:, 1:ROWS + 1].rearrange("p (a b) -> p a b", b=3)[:, :, 0:2],
                a_T[:, 0:ROWS:3][:, :, None].broadcast_to([128, SC, 2]))
            # block-diag masked stationaries [128, 192]
            s1_big = wpool.tile([128, SC * ROWS], BF16, tag="s1_big")
            nc.vector.tensor_mul(
                s1_big[:].rearrange("p (a b) -> p a b", b=ROWS),
                s1[:, 0:ROWS][:, None, :].broadcast_to([128, SC, ROWS]),
                bdiag.rearrange("p (a b) -> p a b", b=ROWS))
            s2_big = wpool.tile([128, SC * ROWS], BF16, tag="s2_big")
            nc.vector.tensor_mul(
                s2_big[:].rearrange("p (a b) -> p a b", b=ROWS),
                a_T[:, 2:ROWS + 2][:, None, :].broadcast_to([128, SC, ROWS]),
                bdiag.rearrange("p (a b) -> p a b", b=ROWS))

            # valid mask: zero the event-0 column (after a_sb consumed m_sb)
            nc.vector.memset(im[:, E:E + 1], 0.0)

            # eqd_m[(s,h), 128h+e] = exp(-D*t[s,h,e]) * valid, 0 off-diagonal
            eqd = wpool.tile([ROWS, P * E], F32, tag="eqd")
            nc.scalar.activation(
                eqd[:].rearrange("p (a b) -> p a b", b=E),
                t_sb[:, None, :].broadcast_to([ROWS, P, E]),
                AF.Exp, scale=-DECAY)
            vd = wpool.tile([ROWS, P * E], F32, tag="vd")
            nc.vector.tensor_mul(
                vd[:].rearrange("p (a b) -> p a b", b=E),
                m_sb[:, None, :].broadcast_to([ROWS, P, E]),
                rdiag.rearrange("p (a b) -> p a b", b=E))
            nc.vector.tensor_mul(eqd[:], eqd[:], vd[:])

            # ---- integral ALU chain on GpSimd (idle engine) ----
            u = wpool.tile([128, ROWS], F32, tag="u")
            nc.vector.tensor_scalar(u[:], tp_T[:], 1.0 / RES, None, ALU.mult)
            ci_t = wpool.tile([128, ROWS], I32, tag="ci_t")
            nc.vector.tensor_copy(ci_t[:], u[:])
            cf_t = wpool.tile([128, ROWS], F32, tag="cf_t")
            nc.vector.tensor_copy(cf_t[:], ci_t[:])
            gt_t = wpool.tile([128, ROWS], F32, tag="gt_t")
            nc.vector.tensor_tensor(gt_t[:], cf_t[:], u[:], ALU.is_gt)
            nc.vector.tensor_sub(gt_t[:], cf_t[:], gt_t[:])  # floor(u)
            ie = wpool.tile([128, ROWS], F32, tag="ie")
            nc.scalar.activation(ie[:], gt_t[:], AF.Exp, scale=-DECAY * RES,
                                 bias=-DECAY * RES)
            nc.vector.tensor_scalar(ie[:], ie[:], E2C, 0.0, ALU.subtract,
                                    ALU.max)
            cm = wpool.tile([128, ROWS], F32, tag="cm")
            nc.vector.tensor_mul(cm[:], ie[:], a_ps[:])
            kint_ps = psI.tile([ROWS, 1], F32, tag="kint")
            nc.tensor.matmul(kint_ps[:], cm[:], ones_col, start=True, stop=True)


            # ---- compares + matvecs, interleaved per sample ----
            # per-sample 512 col block of C: [c1 | c0a c0b | c2]
            call = cmpool.tile([128, SC * 4 * E], BF16, tag="call")
            kk_ps = psK.tile([ROWS, P * E], F32, tag="kk")

            for s in range(SC):
                base = 4 * E * s
                q0 = tq16[:, P * E * s:P * E * s + E]        # pred-0 queries
                q12 = tq16[:, P * E * s + E:P * E * (s + 1)]  # pred-1,2 queries
                nc.vector.tensor_scalar(
                    call[:, base:base + E], q0,
                    tp_T[:, 3 * s + 1:3 * s + 2], 0.0, ALU.subtract, ALU.is_gt)
                nc.vector.tensor_scalar(
                    call[:, base + E:base + 3 * E], q12,
                    tp_T[:, 3 * s:3 * s + 1], 0.0, ALU.subtract, ALU.is_gt)
                nc.vector.tensor_scalar(
                    call[:, base + 3 * E:base + 4 * E], q0,
                    tp_T[:, 3 * s + 2:3 * s + 3], 0.0, ALU.subtract, ALU.is_gt)
                nc.tensor.matmul(
                    kk_ps[:], s1_big[:, ROWS * s:ROWS * (s + 1)],
                    call[:, base:base + 3 * E],
                    start=(s == 0), stop=False, skip_group_check=True)
                nc.tensor.matmul(
                    kk_ps[:, 0:E], s2_big[:, ROWS * s:ROWS * (s + 1)],
                    call[:, base + 3 * E:base + 4 * E],
                    start=False, stop=(s == SC - 1), skip_group_check=True)

            # ---- epilogue: keq -> ln(w*keq+b) with free accum ----
            keq = wpool.tile([ROWS, P * E], F32, tag="keq")
            nc.vector.tensor_mul(keq[:], kk_ps[:], eqd[:])
            lnr = wpool.tile([ROWS, P * E], F32, tag="lnr")
            acc = wpool.tile([ROWS, 2], F32, tag="acc")
            nc.vector.tensor_copy(acc[:, 1:2], kint_ps[:])
            nc.scalar.activation(lnr[:], keq[:], AF.Ln, bias=b_col,
                                 scale=w_col, accum_out=acc[:, 0:1])
            nc.sync.dma_start(out_d[:], acc[:])

    nc.compile()
    _unify_act_tables(nc)
    return nc


def _unify_act_tables(nc):
    for blk in nc.m.functions[0].blocks:
        loads = [i for i in blk.instructions
                 if isinstance(i, mybir.InstLoadActFuncSet)]
        if not loads:
            continue
        loads[0].act_func_set_id = _ACT_SET_ALL
        for ins in loads[1:]:
            blk.instructions.remove(ins)


_NC = None


def _get_nc():
    global _NC
    if _NC is None:
        _NC = _build_nc()
    return _NC


def make_in_maps(event_times, event_mask, base, weight):
    et = np.ascontiguousarray(np.asarray(event_times, np.float32))
    mk = np.ascontiguousarray(np.asarray(event_mask, np.float32))
    w = np.asarray(weight, np.float32).reshape(P)
    b = np.asarray(base, np.float32).reshape(P)
    in_maps = []
    for c in range(N_CORES):
        et_c = et[c * SC:(c + 1) * SC].reshape(ROWS, E)
        em = np.zeros((ROWS, 2 * E + 2), np.float32)
        em[:, 0:E] = et_c
        em[:, E:2 * E] = mk[c * SC:(c + 1) * SC].reshape(ROWS, E)
        em[:, 2 * E] = np.tile(w, SC)
        em[:, 2 * E + 1] = np.tile(b, SC)
        in_maps.append({"em": em, "t16": et_c.astype(np.float16)})
    return in_maps


def host_const(event_mask, base):
    """-RES*G*S*sum(b)  minus the ln(b) contributions of dead cells."""
    b = np.asarray(base, np.float64).reshape(P)
    mk = np.asarray(event_mask, np.float64)
    v_cnt = mk[:, :, 1:].sum(axis=2)              # [S, P] valid counts
    junk = ((P * E - v_cnt) * np.log(b)[None, :]).sum()
    return float(-RES * G * S * b.sum() - junk)


LAST_RESULT = None


def kernel(event_times, event_mask, base, weight, T_max=50, _trace=False, **_):
    global LAST_RESULT
    nc = _get_nc()
    in_maps = make_in_maps(event_times, event_mask, base, weight)
    kwargs = {}
    if _trace:
        kwargs = dict(trace=True, trace_cores=list(range(N_CORES)))
    res = run_bass_kernel_spmd(nc, in_maps, core_ids=list(range(N_CORES)),
                               **kwargs)
    LAST_RESULT = res
    w = np.asarray(weight, np.float64).reshape(P)
    v = -RES * INV1MR * (BODY.T @ w)          # [P]
    v24 = np.tile(v, SC)
    total = np.float64(0.0)
    for r in res.results:
        out = np.asarray(r["out"], np.float64)
        total += out[:, 0].sum() + (out[:, 1] * v24).sum()
    total += host_const(event_mask, base)
    return np.asarray(total, dtype=np.float32)


# revision 10
# speedup vs baseline: 1.0560x; 1.0094x over previous
"""Trainium2 Bass kernel for the logic-model log-likelihood (v3).

Changes vs v2 (driven by the v2 NTFF trace):
  - No dummy activation and no const-block bias columns: the framework's
    own memset consts serve activation biases, so nothing on the Act
    engine waits for the const DMA. The single (rewritten) act-table
    load sits first in the Act program and runs at t~0.
  - Query times are broadcast as fp16 (half the DMA bytes; host supplies
    the fp16 copy), and compares run as 24 tensor_scalar ops in DVE 2x
    mode, interleaved per-sample with the matvec matmuls.
  - DMAs: small inputs first, split across SP/Pool/DVE issue queues.
  - Integral ALU chain runs on the otherwise-idle GpSimd engine.
  - Epilogue: eqd is pre-masked with the valid mask, so dead cells give
    ln(b); the host subtracts sum((384-V)*ln(b)) exactly. Path after the
    last matmul is just mult -> Ln(accum) -> 2 tiny matmuls -> DMA (from
    PSUM directly).
"""
import sys

import numpy as np

sys.path.insert(0, "/opt/trn_rl_repo")

import concourse.bacc as bacc
import concourse.mybir as mybir
from concourse import tile
from concourse.bass_utils import run_bass_kernel_spmd

F32 = mybir.dt.float32
F16 = mybir.dt.float16
I32 = mybir.dt.int32
BF16 = mybir.dt.bfloat16
AF = mybir.ActivationFunctionType
ALU = mybir.AluOpType

N_CORES = 8
S, P, E = 64, 3, 128
SC = S // N_CORES          # samples per core
ROWS = SC * P              # 24 (s,p) rows per core
DECAY, RES, TOL = 0.8, 0.03, 0.1
G = 1667                   # len(np.arange(0, 50, 0.03))
INV1MR = float(1.0 / (1.0 - np.exp(-DECAY * RES)))
E2C = float(np.exp(-DECAY * G * RES))
BODY = np.array([[0, 1, 1], [1, 0, 0], [1, 0, 0]], dtype=np.float32)

# natural_log_exp_and_others: exp, ln, copy, relu in one table
_ACT_SET_ALL = 6


def _cblk():
    # [128, 218]: eye24 | ones | -D*RES | bdiag[192]
    cb = np.zeros((128, 218), np.float32)
    cb[0:ROWS, 0:ROWS] = np.eye(ROWS)
    cb[:, ROWS] = 1.0
    cb[:, ROWS + 1] = -DECAY * RES
    bd = np.zeros((SC, ROWS), np.float32)
    for s in range(SC):
        bd[s, 3 * s:3 * s + 3] = 1.0
    cb[:, 26:26 + SC * ROWS] = bd.reshape(1, SC * ROWS)
    return cb


def _rdiag():
    rd = np.zeros((ROWS, P, E), np.float32)
    for r in range(ROWS):
        rd[r, r % 3, :] = 1.0
    return rd.reshape(ROWS, P * E)


def _build_nc():
    nc = bacc.Bacc(None, target_bir_lowering=False)
    em_d = nc.dram_tensor("em", [ROWS, 2 * E + 2], F32, kind="ExternalInput")
    t16_d = nc.dram_tensor("t16", [ROWS, E], F16, kind="ExternalInput")
    out_d = nc.dram_tensor("out", [ROWS, 2], F32, kind="ExternalOutput")
    cb_d = nc.inline_tensor(_cblk(), "cblk")
    rd_d = nc.inline_tensor(_rdiag(), "rdiag")
    t16_flat = t16_d[:].rearrange("a b -> (a b)")

    with tile.TileContext(nc) as tc:
        with (
            tc.tile_pool(name="const", bufs=1) as cpool,
            tc.tile_pool(name="inp", bufs=1) as ipool,
            tc.tile_pool(name="tq", bufs=1) as qpool,
            tc.tile_pool(name="cmp", bufs=1) as cmpool,
            tc.tile_pool(name="work", bufs=1) as wpool,
            tc.tile_pool(name="psT", bufs=1, space="PSUM") as psT,
            tc.tile_pool(name="psA", bufs=1, space="PSUM") as psA,
            tc.tile_pool(name="psK", bufs=1, space="PSUM") as psK,
            tc.tile_pool(name="psI", bufs=1, space="PSUM") as psI,
            tc.tile_pool(name="psD", bufs=1, space="PSUM") as psD,
        ):
            # ---- DMAs: broadcasts split by partition ranges so their
            # per-partition descriptors land on parallel queues ----
            im = ipool.tile([ROWS, 2 * E + 2], F32, tag="im")
            nc.sync.dma_start(im[:], em_d[:])
            tq16 = qpool.tile([128, SC * P * E], F16, tag="tq16")
            ha = 4 * P * E
            for lo, hi in ((0, 32), (32, 64)):
                nc.sync.dma_start(
                    tq16[lo:hi, 0:ha],
                    t16_flat[0:ha].partition_broadcast(hi - lo))
            for lo, hi in ((64, 96), (96, 128)):
                nc.scalar.dma_start(
                    tq16[lo:hi, 0:ha],
                    t16_flat[0:ha].partition_broadcast(hi - lo))
            cblk = cpool.tile([128, 218], F32, tag="cblk")
            nc.gpsimd.dma_start(cblk[:], cb_d[:])
            for lo, hi in ((0, 64), (64, 128)):
                nc.gpsimd.dma_start(
                    tq16[lo:hi, ha:2 * ha],
                    t16_flat[ha:2 * ha].partition_broadcast(hi - lo))
            rdiag_t = cpool.tile([ROWS, P * E], F32, tag="rdiag")
            nc.gpsimd.dma_start(rdiag_t[:], rd_d[:])

            t_sb = im[:, 0:E]
            m_sb = im[:, E:2 * E]
            w_col = im[:, 2 * E:2 * E + 1]
            b_col = im[:, 2 * E + 1:2 * E + 2]
            eye24 = cblk[0:ROWS, 0:ROWS]
            ones_col = cblk[:, ROWS:ROWS + 1]
            nc.const_aps.aps[(F32, -DECAY * RES)] = \
                cblk[:, ROWS + 1:ROWS + 2]
            bdiag = cblk[:, 26:26 + SC * ROWS]
            rdiag = rdiag_t[:]

            # ---- prep ----
            aexp = wpool.tile([ROWS, E], F32, tag="aexp")
            nc.scalar.activation(aexp[:], t_sb, AF.Exp, scale=DECAY)
            a_sb = wpool.tile([ROWS, E], F32, tag="a_sb")
            nc.vector.tensor_mul(a_sb[:], aexp[:], m_sb)

            t_ps = psT.tile([128, ROWS], F32, tag="t_ps")
            nc.tensor.transpose(t_ps[:], t_sb, eye24)
            tp_T = wpool.tile([128, ROWS], F32, tag="tp_T")  # t^T + TOL
            nc.vector.tensor_scalar(tp_T[:], t_ps[:], TOL, None, ALU.add)

            a_ps = psA.tile([128, ROWS], F32, tag="a_ps")
            nc.tensor.transpose(a_ps[:], a_sb[:], eye24)
            a_T = wpool.tile([128, ROWS + 2], BF16, tag="a_T")
            nc.vector.tensor_copy(a_T[:, 0:ROWS], a_ps[:])
            nc.vector.memset(a_T[:, ROWS:ROWS + 2], 0.0)

            # S1 gather: col 3s <- a[s,1]; cols 3s+1,3s+2 <- a[s,0]
            s1 = wpool.tile([128, ROWS + 1], BF16, tag="s1")
            nc.vector.tensor_copy(s1[:, 0:ROWS:3], a_T[:, 1:ROWS:3])
            nc.vector.tensor_copy(
                s1[:, 1:ROWS + 1].rearrange("p (a b) -> p a b", b=3)[:, :, 0:2],
                a_T[:, 0:ROWS:3][:, :, None].broadcast_to([128, SC, 2]))
            # block-diag masked stationaries [128, 192]
            s1_big = wpool.tile([128, SC * ROWS], BF16, tag="s1_big")
            nc.vector.tensor_mul(
                s1_big[:].rearrange("p (a b) -> p a b", b=ROWS),
                s1[:, 0:ROWS][:, None, :].broadcast_to([128, SC, ROWS]),
                bdiag.rearrange("p (a b) -> p a b", b=ROWS))
            s2_big = wpool.tile([128, SC * ROWS], BF16, tag="s2_big")
            nc.vector.tensor_mul(
                s2_big[:].rearrange("p (a b) -> p a b", b=ROWS),
                a_T[:, 2:ROWS + 2][:, None, :].broadcast_to([128, SC, ROWS]),
                bdiag.rearrange("p (a b) -> p a b", b=ROWS))

            # valid mask: zero the event-0 column (after a_sb consumed m_sb)
            nc.vector.memset(im[:, E:E + 1], 0.0)

            # eqd_m[(s,h), 128h+e] = exp(-D*t[s,h,e]) * valid, 0 off-diagonal
            eqd = wpool.tile([ROWS, P * E], F32, tag="eqd")
            nc.scalar.activation(
                eqd[:].rearrange("p (a b) -> p a b", b=E),
                t_sb[:, None, :].broadcast_to([ROWS, P, E]),
                AF.Exp, scale=-DECAY)
            vd = wpool.tile([ROWS, P * E], F32, tag="vd")
            nc.gpsimd.tensor_mul(
                vd[:].rearrange("p (a b) -> p a b", b=E),
                m_sb[:, None, :].broadcast_to([ROWS, P, E]),
                rdiag.rearrange("p (a b) -> p a b", b=E))
            nc.gpsimd.tensor_mul(eqd[:], eqd[:], vd[:])

            # ---- integral ALU chain on GpSimd (idle engine) ----
            u = wpool.tile([128, ROWS], F32, tag="u")
            nc.vector.tensor_scalar(u[:], tp_T[:], 1.0 / RES, None, ALU.mult)
            ci_t = wpool.tile([128, ROWS], I32, tag="ci_t")
            nc.vector.tensor_copy(ci_t[:], u[:])
            cf_t = wpool.tile([128, ROWS], F32, tag="cf_t")
            nc.vector.tensor_copy(cf_t[:], ci_t[:])
            gt_t = wpool.tile([128, ROWS], F32, tag="gt_t")
            nc.vector.tensor_tensor(gt_t[:], cf_t[:], u[:], ALU.is_gt)
            nc.vector.tensor_sub(gt_t[:], cf_t[:], gt_t[:])  # floor(u)
            ie = wpool.tile([128, ROWS], F32, tag="ie")
            nc.scalar.activation(ie[:], gt_t[:], AF.Exp, scale=-DECAY * RES,
                                 bias=-DECAY * RES)
            nc.vector.tensor_scalar(ie[:], ie[:], E2C, 0.0, ALU.subtract,
                                    ALU.max)
            cm = wpool.tile([128, ROWS], F32, tag="cm")
            nc.vector.tensor_mul(cm[:], ie[:], a_ps[:])
            kint_ps = psI.tile([ROWS, 1], F32, tag="kint")
            nc.tensor.matmul(kint_ps[:], cm[:], ones_col, start=True, stop=True)


            # ---- compares + matvecs, interleaved per sample ----
            # per-sample 512 col block of C: [c1 | c0a c0b | c2]
            call = cmpool.tile([128, SC * 4 * E], BF16, tag="call")
            kk_ps = psK.tile([ROWS, P * E], F32, tag="kk")

            for s in range(SC):
                base = 4 * E * s
                q0 = tq16[:, P * E * s:P * E * s + E]        # pred-0 queries
                q12 = tq16[:, P * E * s + E:P * E * (s + 1)]  # pred-1,2 queries
                nc.vector.tensor_scalar(
                    call[:, base:base + E], q0,
                    tp_T[:, 3 * s + 1:3 * s + 2], 0.0, ALU.subtract, ALU.is_gt)
                nc.vector.tensor_scalar(
                    call[:, base + E:base + 3 * E], q12,
                    tp_T[:, 3 * s:3 * s + 1], 0.0, ALU.subtract, ALU.is_gt)
                nc.vector.tensor_scalar(
                    call[:, base + 3 * E:base + 4 * E], q0,
                    tp_T[:, 3 * s + 2:3 * s + 3], 0.0, ALU.subtract, ALU.is_gt)
                nc.tensor.matmul(
                    kk_ps[:], s1_big[:, ROWS * s:ROWS * (s + 1)],
                    call[:, base:base + 3 * E],
                    start=(s == 0), stop=False, skip_group_check=True)
                nc.tensor.matmul(
                    kk_ps[:, 0:E], s2_big[:, ROWS * s:ROWS * (s + 1)],
                    call[:, base + 3 * E:base + 4 * E],
                    start=False, stop=(s == SC - 1), skip_group_check=True)

            # ---- epilogue: keq -> ln(w*keq+b) with free accum ----
            keq = wpool.tile([ROWS, P * E], F32, tag="keq")
            lnr = wpool.tile([ROWS, P * E], F32, tag="lnr")
            acc = wpool.tile([ROWS, 2], F32, tag="acc")
            nc.vector.tensor_copy(acc[:, 1:2], kint_ps[:])
            nc.vector.tensor_mul(keq[:, E:], kk_ps[:, E:], eqd[:, E:])
            nc.vector.tensor_mul(keq[:, 0:E], kk_ps[:, 0:E], eqd[:, 0:E])
            nc.scalar.activation(lnr[:], keq[:], AF.Ln, bias=b_col,
                                 scale=w_col, accum_out=acc[:, 0:1])
            nc.sync.dma_start(out_d[:], acc[:])

    nc.compile()
    _unify_act_tables(nc)
    return nc


def _unify_act_tables(nc):
    for blk in nc.m.functions[0].blocks:
        loads = [i for i in blk.instructions
                 if isinstance(i, mybir.InstLoadActFuncSet)]
        if not loads:
            continue
        loads[0].act_func_set_id = _ACT_SET_ALL
        for ins in loads[1:]:
            blk.instructions.remove(ins)


_NC = None


def _get_nc():
    global _NC
    if _NC is None:
        _NC = _build_nc()
    return _NC


def make_in_maps(event_times, event_mask, base, weight):
    et = np.ascontiguousarray(np.asarray(event_times, np.float32))
    mk = np.ascontiguousarray(np.asarray(event_mask, np.float32))
    w = np.asarray(weight, np.float32).reshape(P)
    b = np.asarray(base, np.float32).reshape(P)
    in_maps = []
    for c in range(N_CORES):
        et_c = et[c * SC:(c + 1) * SC].reshape(ROWS, E)
        em = np.zeros((ROWS, 2 * E + 2), np.float32)
        em[:, 0:E] = et_c
        em[:, E:2 * E] = mk[c * SC:(c + 1) * SC].reshape(ROWS, E)
        em[:, 2 * E] = np.tile(w, SC)
        em[:, 2 * E + 1] = np.tile(b, SC)
        in_maps.append({"em": em, "t16": et_c.astype(np.float16)})
    return in_maps


def host_const(event_mask, base):
    """-RES*G*S*sum(b)  minus the ln(b) contributions of dead cells."""
    b = np.asarray(base, np.float64).reshape(P)
    mk = np.asarray(event_mask, np.float64)
    v_cnt = mk[:, :, 1:].sum(axis=2)              # [S, P] valid counts
    junk = ((P * E - v_cnt) * np.log(b)[None, :]).sum()
    return float(-RES * G * S * b.sum() - junk)


LAST_RESULT = None


def kernel(event_times, event_mask, base, weight, T_max=50, _trace=False, **_):
    global LAST_RESULT
    nc = _get_nc()
    in_maps = make_in_maps(event_times, event_mask, base, weight)
    kwargs = {}
    if _trace:
        kwargs = dict(trace=True, trace_cores=list(range(N_CORES)))
    res = run_bass_kernel_spmd(nc, in_maps, core_ids=list(range(N_CORES)),
                               **kwargs)
    LAST_RESULT = res
    w = np.asarray(weight, np.float64).reshape(P)
    v = -RES * INV1MR * (BODY.T @ w)          # [P]
    v24 = np.tile(v, SC)
    total = np.float64(0.0)
    for r in res.results:
        out = np.asarray(r["out"], np.float64)
        total += out[:, 0].sum() + (out[:, 1] * v24).sum()
    total += host_const(event_mask, base)
    return np.asarray(total, dtype=np.float32)
